# revision 1
# baseline (speedup 1.0000x reference)
"""Trainium2 Bass kernel for an 8x[1024,768] GPT-2-style transformer block.

Sharding: data-parallel — one batch element per NeuronCore (B=8, n_cores=8),
no collectives. Each core runs the full block on its [1024, 768] slice.

Device dataflow (per core):
  A: load x token-major, LN1 stats (mean/var via DVE+ACT), normalize,
     PE-transpose h -> hT feature-major (f32r).
  B: QKV: qkT[1536,1024] = Wqk^T-blocks @ hT (fp32r matmuls, PSUM accum over
     D-chunks), bias per-partition via DVE; v token-major with bias via K=1
     ones-matmul, packed into v_aug[128,12*(64+1)] with a ones column per head
     (yields softmax denominators for free in the o-matmul).
  C: per head: S^T[k,q] = kT^T@qT (PE), P^T = exp(0.125*S^T) (ACT, no
     max-subtraction — scores are bounded), o^T[65,512] = v_aug^T @ P^T
     accumulated over k-chunks (row 64 = sum of exp = denominator),
     normalize via DVE with a K=1 broadcast matmul of the reciprocal.
  D: proj token-major + residual (in-place into x tiles).
  E: LN2 -> h2T (PE transposes).
  F: fc1 feature-major + fused hardware gelu_apprx_tanh (matches the
     reference's tanh gelu formula) with per-partition bias.
  G: fc2 feature-major, per-partition bias, PE-transpose back to token-major,
     add residual, DMA out.

Host folds LN gamma/beta into the weights (W' = g*W, b' = b@W + bias) so the
device LN is a pure normalize, and pre-blocks weights [c,m,128,128] so every
weight block is DMA'd contiguously exactly once.

pad_mask is accepted but unused: it is all-ones per the problem spec, and the
reference's masking of whole query rows with -inf would produce NaN rows
(softmax of all -inf), which cannot occur for the given inputs.
"""

import numpy as np

import concourse.bass as bass
import concourse.mybir as mybir
from concourse import bacc
from concourse.tile import TileContext
from concourse.bass import ts
from concourse.bass_utils import run_bass_kernel_spmd

F32 = mybir.dt.float32
F32R = mybir.dt.float32r
AFT = mybir.ActivationFunctionType
OP = mybir.AluOpType
AX = mybir.AxisListType

P = 128
N = 1024          # tokens per core
D = 768
H = 12
HD = 64
DFF = 3072
NT = N // P       # 8 token tiles
DC = D // P       # 6 feature chunks
QKC = (2 * D) // P  # 12 chunks of q|k features
FFC = DFF // P    # 24 ff chunks
ZC = DC
QB = 512          # q block width
EPS = 1e-5
B = 8


def _make_identity(nc, ident):
    nc.gpsimd.memset(ident[:], 0.0)
    nc.gpsimd.affine_select(
        out=ident[:],
        in_=ident[:],
        compare_op=OP.not_equal,
        fill=1.0,
        base=0,
        pattern=[[-1, P]],
        channel_multiplier=1,
    )


def _layernorm_stats(nc, pools, x_tiles, tag, eps_sb):
    """Per-token mean/rstd for 8 token-major [128, 768] tiles.

    Returns (mu, rstd) as [128, NT] tiles (column t = token tile t)."""
    pstat, psc = pools
    s1 = pstat.tile([P, NT], F32, tag=f"s1{tag}")
    s2 = pstat.tile([P, NT], F32, tag=f"s2{tag}")
    mu = pstat.tile([P, NT], F32, tag=f"mu{tag}")
    tmp = pstat.tile([P, NT], F32, tag=f"tm{tag}")
    rsd = pstat.tile([P, NT], F32, tag=f"rs{tag}")
    for t in range(NT):
        c = slice(t, t + 1)
        sq = psc.tile([P, D], F32, tag="sq")
        nc.vector.scalar_tensor_tensor(
            sq[:], x_tiles[t][:], 1.0, x_tiles[t][:], OP.mult, OP.mult,
            accum_out=s2[:, c],
        )
        nc.vector.reduce_sum(s1[:, c], x_tiles[t][:], axis=AX.X)
        nc.vector.tensor_scalar_mul(mu[:, c], s1[:, c], 1.0 / D)
        nc.vector.tensor_scalar_mul(s2[:, c], s2[:, c], 1.0 / D)
        nc.vector.tensor_tensor(tmp[:, c], mu[:, c], mu[:, c], OP.mult)
        nc.vector.tensor_tensor(s2[:, c], s2[:, c], tmp[:, c], OP.subtract)
        # rstd = exp(-0.5 * ln(var + eps)); avoids the banned ScalarE Rsqrt
        # and stays on the natural_log_exp table set the attention exps use.
        nc.scalar.activation(tmp[:, c], s2[:, c], AFT.Ln, bias=eps_sb[:])
        nc.scalar.activation(rsd[:, c], tmp[:, c], AFT.Exp, scale=-0.5)
    return mu, rsd


def _build():
    nc = bacc.Bacc("TRN2", target_bir_lowering=False, debug=False)

    x_d = nc.dram_tensor("x", [N, D], F32, kind="ExternalInput")
    wqk_d = nc.dram_tensor("wqk", [QKC, DC, P, P], F32, kind="ExternalInput")
    wv_d = nc.dram_tensor("wv", [D, D], F32, kind="ExternalInput")
    wp_d = nc.dram_tensor("wp", [D, D], F32, kind="ExternalInput")
    wf1_d = nc.dram_tensor("wf1", [FFC, DC, P, P], F32, kind="ExternalInput")
    wf2_d = nc.dram_tensor("wf2", [ZC, FFC, P, P], F32, kind="ExternalInput")
    bqk_d = nc.dram_tensor("bqk", [P, QKC], F32, kind="ExternalInput")
    bv_d = nc.dram_tensor("bv", [1, D], F32, kind="ExternalInput")
    bp_d = nc.dram_tensor("bp", [1, D], F32, kind="ExternalInput")
    bf1_d = nc.dram_tensor("bf1", [P, FFC], F32, kind="ExternalInput")
    bf2_d = nc.dram_tensor("bf2", [P, ZC], F32, kind="ExternalInput")
    out_d = nc.dram_tensor("out", [N, D], F32, kind="ExternalOutput")

    from contextlib import ExitStack
    with TileContext(nc) as tc, ExitStack() as L0:
        pc = L0.enter_context(tc.tile_pool(name="consts", bufs=1))
        psc = L0.enter_context(tc.tile_pool(name="scratch", bufs=1))
        pstat = L0.enter_context(tc.tile_pool(name="stats", bufs=1))
        pblk = L0.enter_context(tc.tile_pool(name="outblk", bufs=4))
        # Global PSUM pools shared by every phase (8 banks total), so phase
        # boundaries never serialize on PSUM pool release/realloc.
        psumB = L0.enter_context(tc.tile_pool(name="psB", bufs=2,
                                              space="PSUM"))
        psumS = L0.enter_context(tc.tile_pool(name="psS", bufs=4,
                                              space="PSUM"))

        def big_psum(shape, name):
            return psumB.tile(shape, F32, tag="big", name=name)

        def small_psum(shape, name):
            return psumS.tile(shape, F32, tag="small", name=name)

        # ---- constants ----
        ident = pc.tile([P, P], F32, tag="ident")
        _make_identity(nc, ident)
        ones_f = pc.tile([1, P], F32, tag="ones_f")
        nc.vector.memset(ones_f[:], 1.0)
        ones_r = pc.tile([1, P], F32R, tag="ones_r")
        nc.vector.tensor_copy(ones_r[:], ones_f[:])
        ones_aug = pc.tile([P, H], F32, tag="ones_aug")
        nc.vector.memset(ones_aug[:], 1.0)
        # head-row selector: sel[j, h*HD+m] = (j == h), used to broadcast
        # denominator rows across 64 partitions via a K=H matmul
        sel_f = pc.tile([H, H * HD], F32, tag="sel_f")
        nc.gpsimd.memset(sel_f[:], 1.0)
        nc.gpsimd.affine_select(
            out=sel_f[:], in_=sel_f[:], compare_op=OP.is_ge, fill=0.0,
            base=0, pattern=[[1, H * HD]], channel_multiplier=-HD)
        nc.gpsimd.affine_select(
            out=sel_f[:], in_=sel_f[:], compare_op=OP.is_ge, fill=0.0,
            base=(HD - 1), pattern=[[-1, H * HD]], channel_multiplier=HD)
        sel_r = pc.tile([H, H * HD], F32R, tag="sel_r")
        nc.vector.tensor_copy(sel_r[:], sel_f[:])
        bqk_sb = pc.tile([P, QKC], F32, tag="bqk")
        nc.sync.dma_start(bqk_sb[:], bqk_d.ap())
        bf1_sb = pc.tile([P, FFC], F32, tag="bf1")
        nc.sync.dma_start(bf1_sb[:], bf1_d.ap())
        bf2_sb = pc.tile([P, ZC], F32, tag="bf2")
        nc.sync.dma_start(bf2_sb[:], bf2_d.ap())
        bv_sb = pc.tile([1, D], F32R, tag="bv")
        nc.sync.dma_start(bv_sb[:], bv_d.ap().bitcast(F32R))
        bp_sb = pc.tile([1, D], F32R, tag="bp")
        nc.sync.dma_start(bp_sb[:], bp_d.ap().bitcast(F32R))
        eps_sb = pc.tile([P, 1], F32, tag="eps")
        nc.vector.memset(eps_sb[:], EPS)

        with ExitStack() as L1:
            px = L1.enter_context(tc.tile_pool(name="xres", bufs=1))
            x_tiles = []
            for t in range(NT):
                xt = px.tile([P, D], F32, tag=f"x{t}", name=f"x{t}")
                nc.gpsimd.dma_start(xt[:], x_d.ap()[ts(t, P), :])
                x_tiles.append(xt)

            with ExitStack() as L1b:
                poT = L1b.enter_context(tc.tile_pool(name="oT", bufs=1))
                oT = [poT.tile([P, N], F32R, tag=f"oT{c}", name=f"oT{c}")
                      for c in range(DC)]

                with ExitStack() as L2:
                    pqk = L2.enter_context(tc.tile_pool(name="qkT", bufs=1))
                    pv = L2.enter_context(tc.tile_pool(name="vaug", bufs=1))

                    with ExitStack() as L3:
                        phT = L3.enter_context(tc.tile_pool(name="hT",
                                                            bufs=1))
                        ph = L3.enter_context(tc.tile_pool(name="htmp",
                                                           bufs=2))
                        pwv = L3.enter_context(tc.tile_pool(name="wv",
                                                            bufs=1))
                        pwqk = L3.enter_context(tc.tile_pool(name="wqkp",
                                                             bufs=3))

                        # ---- Phase A: LN1 + transpose ----
                        mu1, rsd1 = _layernorm_stats(
                            nc, (pstat, psc), x_tiles, "a", eps_sb)
                        hT = [phT.tile([P, N], F32R, tag=f"hT{c}",
                                       name=f"hT{c}") for c in range(DC)]
                        for t in range(NT):
                            ht = ph.tile([P, D], F32, tag="h", name="h")
                            nc.vector.tensor_scalar(
                                ht[:], x_tiles[t][:],
                                mu1[:, t:t + 1], rsd1[:, t:t + 1],
                                OP.subtract, OP.mult)
                            for c in range(DC):
                                pt = small_psum([P, P], "pt_a")
                                nc.tensor.transpose(
                                    pt[:], ht[:, ts(c, P)], ident[:])
                                nc.vector.tensor_copy(
                                    hT[c][:, ts(t, P)], pt[:])

                        # ---- Phase B: QKV ----
                        qkT = []
                        for m in range(QKC):
                            pq = big_psum([P, N], f"pq{m}")
                            wm = pwqk.tile([P, DC, P], F32R, tag="wqkm",
                                           name="wqkm")
                            nc.sync.dma_start(
                                wm[:],
                                wqk_d.ap()[m].rearrange(
                                    "c p x -> p c x").bitcast(F32R))
                            for c in range(DC):
                                for j in range(2):
                                    nc.tensor.matmul(
                                        pq[:, ts(j, QB)], wm[:, c, :],
                                        hT[c][:, ts(j, QB)],
                                        start=(c == 0), stop=(c == DC - 1))
                            qm = pqk.tile([P, N], F32R, tag=f"qk{m}",
                                          name=f"qk{m}")
                            nc.vector.tensor_scalar_add(
                                qm[:], pq[:], bqk_sb[:, m:m + 1])
                            qkT.append(qm)

                        wv_sb = []
                        for c in range(DC):
                            wt = pwv.tile([P, D], F32R, tag=f"wv{c}",
                                          name=f"wv{c}")
                            nc.sync.dma_start(
                                wt[:], wv_d.ap()[ts(c, P), :].bitcast(F32R))
                            wv_sb.append(wt)
                        va_tiles = []
                        for t in range(NT):
                            pv_ = big_psum([P, N], f"pv{t}")
                            for c in range(DC):
                                for n0, nw in ((0, 512), (512, 256)):
                                    nc.tensor.matmul(
                                        pv_[:, n0:n0 + nw],
                                        hT[c][:, ts(t, P)],
                                        wv_sb[c][:, n0:n0 + nw],
                                        start=(c == 0), stop=False)
                            for n0, nw in ((0, 512), (512, 256)):
                                nc.tensor.matmul(
                                    pv_[:, n0:n0 + nw], ones_r[:],
                                    bv_sb[:, n0:n0 + nw],
                                    start=False, stop=True)
                            va = pv.tile([P, H * (HD + 1)], F32R,
                                         tag=f"va{t}", name=f"va{t}")
                            va3 = va[:].rearrange("p (h c) -> p h c",
                                                  c=HD + 1)
                            nc.vector.tensor_copy(
                                va3[:, :, HD:HD + 1], ones_aug[:][:, :, None])
                            nc.vector.tensor_copy(
                                va3[:, :, 0:HD],
                                pv_[:, 0:D].rearrange("p (h c) -> p h c",
                                                      c=HD))
                            va_tiles.append(va)

                    # proj weights loaded here so phase D can overlap
                    # the tail of attention
                    pwp = L2.enter_context(tc.tile_pool(name="wp", bufs=1))
                    wp_sb = []
                    for c in range(DC):
                        wt = pwp.tile([P, D], F32R, tag=f"wp{c}",
                                      name=f"wp{c}")
                        nc.sync.dma_start(
                            wt[:], wp_d.ap()[ts(c, P), :].bitcast(F32R))
                        wp_sb.append(wt)

                    # ---- Phase C: attention ----
                    with ExitStack() as L3b:
                        pP = L3b.enter_context(tc.tile_pool(name="probs",
                                                            bufs=4))
                        prc = L3b.enter_context(tc.tile_pool(name="recip",
                                                             bufs=4))
                        pden = L3b.enter_context(tc.tile_pool(name="den",
                                                              bufs=1))
                        den = pden.tile([H, N], F32, tag="den", name="den")
                        rden = pden.tile([H, N], F32, tag="rden",
                                         name="rden")
                        rdenr = pden.tile([H, N], F32R, tag="rdenr",
                                          name="rdenr")
                        zq = psc.tile([P, 2 * QB], F32, tag="sq", name="zq")
                        nc.vector.memset(zq[:], 0.0)
                        qblks = []
                        for i in range(2):
                            qb_t = pden.tile([P, 2, QB], F32R,
                                             tag=f"qblk{i}", name=f"qblk{i}")
                            nc.vector.tensor_copy(
                                qb_t[:], zq[:].rearrange(
                                    "p (a b) -> p a b", a=2))
                            qblks.append(qb_t)
                        for qb in range(N // QB):
                            for hp in range(H // 2):
                                heads = (2 * hp, 2 * hp + 1)
                                qc = hp
                                po = {}
                                for h in heads:
                                    po[h] = small_psum([HD + 1, QB],
                                                       f"po{h}")
                                # block-diagonal q: col-block j holds head
                                # j's q rows, other 64 rows stay zero, so S
                                # for both heads runs as full-K=128 matmuls
                                # (keeps the PE HAM warm)
                                qblk = qblks[hp % 2]
                                nc.vector.tensor_copy(
                                    qblk[0:HD, 0, :],
                                    qkT[qc][0:HD, ts(qb, QB)])
                                nc.vector.tensor_copy(
                                    qblk[HD:P, 1, :],
                                    qkT[qc][HD:P, ts(qb, QB)])
                                for kt in range(NT):
                                    ps_ = big_psum([P, 2, QB], "ps")
                                    for j in range(2):
                                        nc.tensor.matmul(
                                            ps_[:, j, :],
                                            qkT[DC + qc][:, ts(kt, P)],
                                            qblk[:, j, :],
                                            start=True, stop=True)
                                    pt_ = pP.tile([P, 2, QB], F32R,
                                                  tag="pt", name="pt")
                                    nc.scalar.activation(
                                        pt_[:], ps_[:], AFT.Exp, scale=0.125)
                                    for j, h in enumerate(heads):
                                        va_lo = h * (HD + 1)
                                        nc.tensor.matmul(
                                            po[h][:],
                                            va_tiles[kt][
                                                :, va_lo:va_lo + HD + 1],
                                            pt_[:, j, :],
                                            start=(kt == 0),
                                            stop=(kt == NT - 1))
                                for h in heads:
                                    lo = (h % 2) * HD
                                    hi = lo + HD
                                    nc.vector.tensor_copy(
                                        oT[qc][lo:hi, ts(qb, QB)],
                                        po[h][0:HD, :])
                                    dstage = prc.tile([1, QB], F32,
                                                      tag="dstage",
                                                      name="dstage")
                                    nc.vector.tensor_copy(
                                        dstage[:], po[h][HD:HD + 1, :])
                                    nc.sync.dma_start(
                                        den[h:h + 1, ts(qb, QB)], dstage[:])
                            # normalize this q-half while the other half's
                            # S/o matmuls keep the PE busy
                            qs = ts(qb, QB)
                            nc.vector.reciprocal(rden[:, qs], den[:, qs])
                            nc.vector.tensor_copy(rdenr[:, qs], rden[:, qs])
                            for h in range(H):
                                qc, qhalf = divmod(h, 2)
                                lo, hi = qhalf * HD, (qhalf + 1) * HD
                                pb_ = small_psum([HD, QB], f"pb{h}")
                                nc.tensor.matmul(
                                    pb_[:], sel_r[:, ts(h, HD)],
                                    rdenr[:, qs], start=True, stop=True)
                                nc.vector.tensor_tensor(
                                    oT[qc][lo:hi, qs], oT[qc][lo:hi, qs],
                                    pb_[:], OP.mult)

                    # ---- Phase D: proj + residual (in-place into x),
                    # with LN2 stats interleaved per tile ----
                    mu2 = pstat.tile([P, NT], F32, tag="mu2b", name="mu2b")
                    rsd2 = pstat.tile([P, NT], F32, tag="rs2b", name="rs2b")
                    for t in range(NT):
                        py_ = big_psum([P, D], f"py{t}")
                        for c in range(DC):
                            for n0, nw in ((0, 512), (512, 256)):
                                nc.tensor.matmul(
                                    py_[:, n0:n0 + nw], oT[c][:, ts(t, P)],
                                    wp_sb[c][:, n0:n0 + nw],
                                    start=(c == 0), stop=False)
                        for n0, nw in ((0, 512), (512, 256)):
                            nc.tensor.matmul(
                                py_[:, n0:n0 + nw], ones_r[:],
                                bp_sb[:, n0:n0 + nw],
                                start=False, stop=True)
                        nc.vector.tensor_tensor(
                            x_tiles[t][:], py_[:], x_tiles[t][:], OP.add)
                        # LN2 stats for this tile, interleaved so the
                        # E-phase transposes can start before proj finishes
                        c2 = slice(t, t + 1)
                        sq = psc.tile([P, D], F32, tag="sq", name="sq")
                        s2b = pstat.tile([P, 1], F32, tag="s2b", name="s2b")
                        s1b = pstat.tile([P, 1], F32, tag="s1b", name="s1b")
                        nc.vector.scalar_tensor_tensor(
                            sq[:], x_tiles[t][:], 1.0, x_tiles[t][:],
                            OP.mult, OP.mult, accum_out=s2b[:])
                        nc.vector.reduce_sum(s1b[:], x_tiles[t][:],
                                             axis=AX.X)
                        nc.vector.tensor_scalar_mul(
                            mu2[:, c2], s1b[:], 1.0 / D)
                        nc.vector.tensor_scalar_mul(s2b[:], s2b[:], 1.0 / D)
                        nc.vector.tensor_tensor(
                            s1b[:], mu2[:, c2], mu2[:, c2], OP.mult)
                        nc.vector.tensor_tensor(
                            s2b[:], s2b[:], s1b[:], OP.subtract)
                        nc.scalar.activation(
                            s2b[:], s2b[:], AFT.Ln, bias=eps_sb[:])
                        nc.scalar.activation(
                            rsd2[:, c2], s2b[:], AFT.Exp, scale=-0.5)

            # ---- Phases E+F+G in one scope (overlap fc1/fc2/out) ----
            with ExitStack() as L1c:
                pg = L1c.enter_context(tc.tile_pool(name="gT", bufs=1))
                ph2T = L1c.enter_context(tc.tile_pool(name="h2T", bufs=1))
                ph2 = L1c.enter_context(tc.tile_pool(name="h2tmp", bufs=2))
                pwf1 = L1c.enter_context(tc.tile_pool(name="wf1p", bufs=3))
                pwf2 = L1c.enter_context(tc.tile_pool(name="wf2p", bufs=2))
                pzT = L1c.enter_context(tc.tile_pool(name="zT", bufs=1))

                gT = [pg.tile([P, N], F32R, tag=f"g{m}", name=f"g{m}")
                      for m in range(FFC)]
                h2T = [ph2T.tile([P, N], F32R, tag=f"h2T{c}",
                                 name=f"h2T{c}") for c in range(DC)]
                for t in range(NT):
                    ht = ph2.tile([P, D], F32, tag="h2", name="h2")
                    nc.vector.tensor_scalar(
                        ht[:], x_tiles[t][:], mu2[:, t:t + 1],
                        rsd2[:, t:t + 1], OP.subtract, OP.mult)
                    for c in range(DC):
                        pt = small_psum([P, P], "pt_e")
                        nc.tensor.transpose(pt[:], ht[:, ts(c, P)],
                                            ident[:])
                        nc.vector.tensor_copy(h2T[c][:, ts(t, P)], pt[:])

                for m in range(FFC):
                    pg_ = big_psum([P, N], f"pg{m}")
                    wm = pwf1.tile([P, DC, P], F32R, tag="wf1m",
                                   name="wf1m")
                    nc.sync.dma_start(
                        wm[:],
                        wf1_d.ap()[m].rearrange(
                            "c p x -> p c x").bitcast(F32R))
                    for c in range(DC):
                        for j in range(2):
                            nc.tensor.matmul(
                                pg_[:, ts(j, QB)], wm[:, c, :],
                                h2T[c][:, ts(j, QB)],
                                start=(c == 0), stop=(c == DC - 1))
                    nc.scalar.activation(
                        gT[m][:], pg_[:], AFT.Gelu_apprx_tanh,
                        bias=bf1_sb[:, m:m + 1])

                for m in range(ZC):
                    pz_ = big_psum([P, N], f"pz{m}")
                    wm = pwf2.tile([P, FFC, P], F32R, tag="wf2m",
                                   name="wf2m")
                    nc.sync.dma_start(
                        wm[:],
                        wf2_d.ap()[m].rearrange(
                            "c p x -> p c x").bitcast(F32R))
                    for c in range(FFC):
                        for j in range(2):
                            nc.tensor.matmul(
                                pz_[:, ts(j, QB)], wm[:, c, :],
                                gT[c][:, ts(j, QB)],
                                start=(c == 0), stop=(c == FFC - 1))
                    zt = pzT.tile([P, N], F32, tag="zt", name="zt")
                    nc.vector.tensor_scalar_add(
                        zt[:], pz_[:], bf2_sb[:, m:m + 1])
                    for t in range(NT):
                        ptz = small_psum([P, P], "ptz")
                        nc.tensor.transpose(ptz[:], zt[:, ts(t, P)],
                                            ident[:])
                        ob = pblk.tile([P, P], F32, tag="ob", name="ob")
                        nc.vector.tensor_tensor(
                            ob[:], ptz[:], x_tiles[t][:, ts(m, P)], OP.add)
                        nc.sync.dma_start(
                            out_d.ap()[ts(t, P), ts(m, P)], ob[:])

    nc.compile()
    return nc



_CACHE = {}


def _get_nc():
    if "nc" not in _CACHE:
        _CACHE["nc"] = _build()
    return _CACHE["nc"]


def _prep_inputs(inputs):
    f = lambda k: np.ascontiguousarray(np.asarray(inputs[k], np.float32))
    x = f("x")
    w_attn, b_attn = f("w_attn"), f("b_attn")
    w_proj, b_proj = f("w_proj"), f("b_proj")
    w_fc, b_fc = f("w_fc"), f("b_fc")
    w_fc2, b_fc2 = f("w_fc2"), f("b_fc2")
    g1, b1 = f("ln1_g"), f("ln1_b")
    g2, b2 = f("ln2_g"), f("ln2_b")

    # Fold LN affine into the consuming weights: (n*g + b) @ W = n @ (g*W) + b@W
    wqk = g1[:, None] * w_attn[:, :2 * D]
    bqk = b1 @ w_attn[:, :2 * D] + b_attn[:2 * D]
    wv = g1[:, None] * w_attn[:, 2 * D:]
    bv = b1 @ w_attn[:, 2 * D:] + b_attn[2 * D:]
    wf1 = g2[:, None] * w_fc
    bf1 = b2 @ w_fc + b_fc

    # m-major blocking: [m, c, 128, 128] so one contiguous DMA per m-chunk
    wqk_b = np.ascontiguousarray(
        wqk.reshape(DC, P, QKC, P).transpose(2, 0, 1, 3))
    wf1_b = np.ascontiguousarray(
        wf1.reshape(DC, P, FFC, P).transpose(2, 0, 1, 3))
    wf2_b = np.ascontiguousarray(
        w_fc2.reshape(FFC, P, ZC, P).transpose(2, 0, 1, 3))

    common = {
        "wqk": wqk_b,
        "wv": np.ascontiguousarray(wv),
        "wp": w_proj,
        "wf1": wf1_b,
        "wf2": wf2_b,
        "bqk": np.ascontiguousarray(bqk.reshape(QKC, P).T),
        "bv": bv.reshape(1, D),
        "bp": b_proj.reshape(1, D),
        "bf1": np.ascontiguousarray(bf1.reshape(FFC, P).T),
        "bf2": np.ascontiguousarray(b_fc2.reshape(ZC, P).T),
    }
    return [dict(common, x=np.ascontiguousarray(x[b])) for b in range(B)]


def run(inputs, trace=False):
    nc = _get_nc()
    in_maps = _prep_inputs(inputs)
    res = run_bass_kernel_spmd(nc, in_maps, core_ids=list(range(B)),
                               trace=trace)
    out = np.stack([r["out"] for r in res.results], axis=0)
    return out.astype(np.float32), res


def kernel(**inputs):
    out, _ = run(inputs, trace=False)
    return out

